# revision 27
# baseline (speedup 1.0000x reference)
"""Trainium2 Bass kernel for the supervoxel erode/edge loss module.

The reference divides a padded [B,X,Y] grid (pad offset 4*sx along x, 4*sy
along y) into 8x8 patches, zeroes the last row/col of the mask channel in
each patch, erodes along both patch axes and sums eroded*edge. The erode
`a*b + (1-a)*a + (1-b)*a` algebraically equals `2a - a^2` with
a = m(i)*m(i+1) (the second operand cancels), and because both the patch
shifts and the patch-boundary zeroing are local, the whole module collapses
to a global elementwise expression on the unpadded grid:

    mt(x,y) = mask[b,x,y,idx] * [(x+4sx)%8 != 7] * [(y+4sy)%8 != 7]
    ax = mt(x,y)*mt(x+1,y); ay = mt(x,y)*mt(x,y+1)   (zero past image edge)
    total = sum_b,x,y ax(2-ax) * ay(2-ay) * edge
    out = loss_old + total / (B * ((X+8)//8) * ((Y+8)//8))

With raw products ax0 = raw(x)raw(x+1), ay0 = raw(x,y)raw(x,y+1) the masks
fold out of the elementwise chain:

    contribution = ax0(2-ax0) * ay0(2-ay0) * edge * R(x) * C(y)

R(x) = [x%8 not in {6-4sx, 7-4sx}] is applied to the final per-row partial
sums, and C(y) = [y%8 not in {6-4sy, 7-4sy}] by restricting the elementwise
ops to the live columns of each 8-group (sy==0), or by one extra multiply.

x-tiles are 121 rows at stride 120 (one-row overlap so the x-neighbor
product never crosses a tile boundary; 120 % 8 == 0 keeps R per-partition
tile-invariant). DMA is the roofline: per-transfer fixed cost serializes on
the queue rings, so mask tiles are loaded two-at-a-time with one
overlapping-window DMA (~3.9 MiB each) and edge as one whole-image DMA.

Per x-tile the compute pipeline is:
    PE    : shifted = S @ v  (S = shift-by-one-row matrix; v = stride-4
            channel view of the mask tile)
    DVE   : ax0 = v*shifted, nx = (ax0-2)*ax0, ny = (ay0-2)*ay0, reduce
    Pool  : ay0 = v*v(y+1), p1 = nx*ny, p2 = p1*edge
    ((a-2)*a = -(a(2-a)); the two negations cancel in p1 = nx*ny.)

Sharding: data-parallel over batch, B/8 images per core on 8 cores; each
core returns a masked partial sum, combined on host (the mean is a single
scalar, so no device collective is needed).
"""

import sys

sys.path.insert(0, "/opt/trn_rl_repo")

import numpy as np

from concourse import bacc, bass, mybir, tile
from concourse.ap import AP
from concourse.bass_utils import run_bass_kernel_spmd

F32 = mybir.dt.float32
N_CORES = 8
TS = 120  # x-tile stride (multiple of 8 so the %8 row pattern is tile-invariant)
SHIFTS = [(0, 0), (1, 0), (0, 1), (1, 1)]


def _build_program(
    Bc: int,
    X: int,
    Y: int,
    idx: int,
    sy: int,
    niter: int = 1,
    variant: str = "full",
    dma_mode: str = "rr3",
):
    """Build the per-core Bass program. Inputs (per core):
    mask [Bc,X,Y,4] f32, edge [Bc,X,Y,1] f32, smat [128,128], rvec [128,1],
    cvec [128,Y] (used only when sy != 0). Output: out [1,1] f32 partial sum.
    niter > 1 repeats the whole computation on-device (timing only).
    """
    assert X % 8 == 0 and Y % 8 == 0
    nk = (X + TS - 1) // TS  # x-tiles per image
    nt = Bc * nk + 1  # total chunks (image 0 is split into one extra chunk)
    npair = nk // 2
    odd_rows = X - 2 * TS * npair  # rows of the trailing unpaired tile (0 if none)
    G = Y // 8
    packed = sy == 0  # live cols are j in 0..5 of every group of 8

    nc = bacc.Bacc("TRN2", target_bir_lowering=False, debug=False)
    mask_h = nc.dram_tensor("mask", [Bc, X, Y, 4], F32, kind="ExternalInput")
    edge_h = nc.dram_tensor("edge", [Bc, X, Y, 1], F32, kind="ExternalInput")
    smat_h = nc.dram_tensor("smat", [128, 128], F32, kind="ExternalInput")
    rvec_h = nc.dram_tensor("rvec", [128, 1], F32, kind="ExternalInput")
    cvec_h = nc.dram_tensor("cvec", [128, Y], F32, kind="ExternalInput")
    out_h = nc.dram_tensor("out", [1, 1], F32, kind="ExternalOutput")

    if dma_mode == "gpsimd":
        eng_mask, eng_edge = "gpsimd", "sync"
    elif dma_mode == "sync":
        eng_mask, eng_edge = "sync", "scalar"
    else:
        eng_mask, eng_edge = "scalar", "sync"

    def mask_pair_src(b, m):
        """Overlapping-window DRAM AP: [121, 2, Y, 4] where element
        (p, j, y, c) reads mask[b, 2*TS*m + TS*j + p, y, c]."""
        row = Y * 4  # elements per x-row
        off = (b * X + 2 * TS * m) * row
        ap = [[row, TS + 1], [TS * row, 2], [4, Y], [1, 4]]
        return AP(mask_h, off, ap)

    def edge_pair_src(b, m):
        """[TS, 2, Y] where (p, j, y) reads edge[b, 2*TS*m + TS*j + p, y, 0]."""
        off = (b * X + 2 * TS * m) * Y
        ap = [[Y, TS], [TS * Y, 2], [1, Y]]
        return AP(edge_h, off, ap)

    def mask_tile_src(b, k):
        """[121, Y, 4]: rows TS*k .. TS*k+121 of image b (one overlap row)."""
        row = Y * 4
        off = (b * X + TS * k) * row
        ap = [[row, TS + 1], [4, Y], [1, 4]]
        return AP(mask_h, off, ap)

    def mask_tile_src_strided(b, k):
        """[121, Y]: channel idx only, 4B elements at 16B stride."""
        row = Y * 4
        off = (b * X + TS * k) * row + idx
        ap = [[row, TS + 1], [4, Y]]
        return AP(mask_h, off, ap)

    with tile.TileContext(nc) as tc:
        with (
            tc.tile_pool(name="mt", bufs=6 if dma_mode.startswith("rr") else 2) as mt_pool,
            tc.tile_pool(name="et", bufs=6 if dma_mode.startswith("rr") else 2) as et_pool,
            tc.tile_pool(name="mtt", bufs=1) as mtt_pool,
            tc.tile_pool(name="ett", bufs=1) as ett_pool,
            tc.tile_pool(name="work", bufs=2) as w_pool,
            tc.tile_pool(name="pp", bufs=2) as p_pool,
            tc.tile_pool(name="psum", bufs=2, space="PSUM") as ps_pool,
            tc.tile_pool(name="psum1", bufs=1, space="PSUM") as ps1_pool,
            tc.tile_pool(name="const", bufs=1) as c_pool,
        ):
            smat_t = c_pool.tile([128, 128], F32)
            rvec_t = c_pool.tile([128, 1], F32)
            ones_t = c_pool.tile([128, 1], F32)
            neg1_t = c_pool.tile([128, 1], F32)
            partials = c_pool.tile([128, nt], F32)
            nc.sync.dma_start(smat_t[:], smat_h.ap())
            nc.sync.dma_start(rvec_t[:], rvec_h.ap())
            nc.gpsimd.memset(ones_t[:], 1.0)
            nc.gpsimd.memset(neg1_t[:], -1.0)
            cvec_t = None
            if not packed:
                cvec_t = c_pool.tile([128, Y], F32)
                nc.sync.dma_start(cvec_t[:], cvec_h.ap())

            def emit_compute(v, et_v, cr, t_idx):
                """v: [rows>=cr(+1), Y] stride-4 mask-channel view;
                et_v: [cr, Y] edge view; accumulates into partials[:, t_idx]."""
                rows = v.shape[0]
                if variant == "dma":
                    # timing ablation: loads only, tiny consumer so nothing is elided
                    nc.vector.reduce_sum(
                        partials[0:1, t_idx : t_idx + 1],
                        v[0:1, 0:8],
                        axis=mybir.AxisListType.X,
                    )
                    nc.gpsimd.tensor_mul(
                        partials[0:1, t_idx : t_idx + 1],
                        partials[0:1, t_idx : t_idx + 1],
                        et_v[0:1, 0:1],
                    )
                    return
                shifted = ps_pool.tile([128, Y], F32)
                if variant != "pool":
                    for c0 in range(0, Y, 512):
                        cw = min(512, Y - c0)
                        nc.tensor.matmul(
                            shifted[:, c0 : c0 + cw],
                            smat_t[0:rows, :],
                            v[:, c0 : c0 + cw],
                            start=True,
                            stop=True,
                        )

                if packed:
                    ax0 = w_pool.tile([cr, G, 6], F32)
                    ay0 = w_pool.tile([cr, G, 6], F32)
                    if variant != "full":
                        nxt = w_pool.tile([cr, G, 6], F32)
                        nyt = w_pool.tile([cr, G, 6], F32)
                    p1 = p_pool.tile([cr, G, 6], F32)
                    p2 = p_pool.tile([cr, G, 6], F32)

                    def lv(t, j0=0, j1=6):
                        return t.rearrange("p (g j) -> p g j", j=8)[:, :, j0:j1]

                    v_l = lv(v[0:cr, :])
                    v_l1 = lv(v[0:cr, :], 1, 7)  # col + 1
                    sh_l = lv(shifted[0:cr, :])
                    if variant == "full":
                        # balanced 5-engine split. With tx=(ax0-1)^2 and
                        # ty=(ay0-1)^2 (Act engine), e_x*e_y = (tx-1)*(ty-1):
                        # the Act Square offload keeps DVE+Pool under the DMA
                        # slot, and the last STT fuses p2 with its reduction
                        # via accum_out.
                        tx = w_pool.tile([cr, G, 6], F32)
                        ty = w_pool.tile([cr, G, 6], F32)
                        u = w_pool.tile([cr, G, 6], F32)
                        nc.vector.tensor_mul(ax0[:], v_l, sh_l)
                        nc.gpsimd.tensor_mul(ay0[:], v_l, v_l1)
                        nc.scalar.activation(
                            tx[:], ax0[:], mybir.ActivationFunctionType.Square,
                            neg1_t[0:cr, :],
                        )
                        nc.scalar.activation(
                            ty[:], ay0[:], mybir.ActivationFunctionType.Square,
                            neg1_t[0:cr, :],
                        )
                        nc.gpsimd.tensor_scalar_sub(u[:], ty[:], 1.0)
                        nc.vector.scalar_tensor_tensor(
                            p1[:], tx[:], 1.0, u[:],
                            op0=mybir.AluOpType.subtract, op1=mybir.AluOpType.mult,
                        )
                        nc.vector.scalar_tensor_tensor(
                            p2[:], p1[:], 1.0, lv(et_v),
                            op0=mybir.AluOpType.mult, op1=mybir.AluOpType.mult,
                            accum_out=partials[0:cr, t_idx : t_idx + 1],
                        )
                        return
                    if variant == "dve":
                        nc.vector.tensor_mul(ax0[:], v_l, sh_l)
                        nc.vector.scalar_tensor_tensor(
                            nxt[:], ax0[:], 2.0, ax0[:],
                            op0=mybir.AluOpType.subtract, op1=mybir.AluOpType.mult,
                        )
                        nc.vector.scalar_tensor_tensor(
                            nyt[:], nxt[:], 2.0, nxt[:],
                            op0=mybir.AluOpType.subtract, op1=mybir.AluOpType.mult,
                        )
                        nc.vector.reduce_sum(
                            partials[0:cr, t_idx : t_idx + 1], nyt[:],
                            axis=mybir.AxisListType.XY,
                        )
                        return
                    if variant == "pool":
                        nc.gpsimd.tensor_mul(ay0[:], v_l, v_l1)
                        nc.gpsimd.tensor_mul(p1[:], ay0[:], ay0[:])
                        nc.gpsimd.tensor_mul(p2[:], p1[:], lv(et_v))
                        nc.vector.reduce_sum(
                            partials[0:cr, t_idx : t_idx + 1], p2[:],
                            axis=mybir.AxisListType.XY,
                        )
                        return
                    # ax0 = v * (v shifted one row); ay0 = v * (v shifted one col)
                    nc.vector.tensor_mul(ax0[:], v_l, sh_l)
                    nc.gpsimd.tensor_mul(ay0[:], v_l, v_l1)
                    # n = (a - 2) * a = -e; the negations cancel in the product
                    nc.vector.scalar_tensor_tensor(
                        nxt[:], ax0[:], 2.0, ax0[:],
                        op0=mybir.AluOpType.subtract, op1=mybir.AluOpType.mult,
                    )
                    nc.vector.scalar_tensor_tensor(
                        nyt[:], ay0[:], 2.0, ay0[:],
                        op0=mybir.AluOpType.subtract, op1=mybir.AluOpType.mult,
                    )
                    nc.gpsimd.tensor_mul(p1[:], nxt[:], nyt[:])
                    nc.gpsimd.tensor_mul(p2[:], p1[:], lv(et_v))
                    nc.vector.reduce_sum(
                        partials[0:cr, t_idx : t_idx + 1], p2[:],
                        axis=mybir.AxisListType.XY,
                    )
                else:
                    W = Y - 1
                    ax0 = w_pool.tile([cr, Y], F32)
                    ay0 = w_pool.tile([cr, Y], F32)
                    nxt = w_pool.tile([cr, Y], F32)
                    nyt = w_pool.tile([cr, Y], F32)
                    p1 = p_pool.tile([cr, Y], F32)
                    p2 = p_pool.tile([cr, Y], F32)
                    nc.vector.tensor_mul(ax0[:, 0:W], v[0:cr, 0:W], shifted[0:cr, 0:W])
                    nc.gpsimd.tensor_mul(ay0[:, 0:W], v[0:cr, 0:W], v[0:cr, 1:Y])
                    # fold the column mask into ay0 (C is 0/1 so e_y picks it up)
                    nc.gpsimd.tensor_mul(ay0[:, 0:W], ay0[:, 0:W], cvec_t[0:cr, 0:W])
                    nc.vector.scalar_tensor_tensor(
                        nxt[:, 0:W], ax0[:, 0:W], 2.0, ax0[:, 0:W],
                        op0=mybir.AluOpType.subtract, op1=mybir.AluOpType.mult,
                    )
                    nc.vector.scalar_tensor_tensor(
                        nyt[:, 0:W], ay0[:, 0:W], 2.0, ay0[:, 0:W],
                        op0=mybir.AluOpType.subtract, op1=mybir.AluOpType.mult,
                    )
                    nc.gpsimd.tensor_mul(p1[:, 0:W], nxt[:, 0:W], nyt[:, 0:W])
                    nc.gpsimd.tensor_mul(p2[:, 0:W], p1[:, 0:W], et_v[:, 0:W])
                    nc.vector.reduce_sum(
                        partials[0:cr, t_idx : t_idx + 1], p2[:, 0:W],
                        axis=mybir.AxisListType.X,
                    )

            rr_state = [0]
            ring_cycle = {
                "rr1": (nc.sync,),
                "rr2": (nc.sync, nc.scalar),
                "rr3": (nc.sync, nc.scalar, nc.gpsimd),
                "rr3s": (nc.sync, nc.scalar, nc.gpsimd),
                "rrs": (nc.sync, nc.scalar, nc.gpsimd),
            }.get(dma_mode, (nc.sync, nc.scalar))
            # rr3s: split packets to 4KB to raise per-queue outstanding count
            dma_kw = {"max_dma_last_dim": 1024} if dma_mode == "rr3s" else {}

            def rr_dma(dst, src):
                """Round-robin over DMA rings. The SP/Activation HWDGE rings
                never stall behind engine work; qPoolDynamic (SWDGE) adds a
                third stream but its descriptor prep shares the Pool engine
                with the gpsimd muls."""
                eng = ring_cycle[rr_state[0] % len(ring_cycle)]
                rr_state[0] += 1
                return eng.dma_start(dst, src, **dma_kw)

            def emit_iter():
                nc.vector.memset(partials[:], 0.0)
                if dma_mode.startswith("rr"):
                    # chunked rows, round-robin across the DMA rings. The
                    # first two chunks of image 0 are small (64 rows) so the
                    # first compute can start ~15us in instead of waiting for
                    # a full 120-row tile to win its share of the rings.
                    # Every chunk start stays == 0 mod 8 (R row-mask phase).
                    t_ctr = [0]
                    for b in range(Bc):
                        if b == 0:
                            chunks = [(0, 64), (64, 64)]
                            chunks += [(128 + TS * i, TS) for i in range(7)]
                            chunks += [(968, X - 968)]
                        else:
                            chunks = [(TS * i, TS) for i in range(2 * npair)]
                            if odd_rows:
                                chunks += [(2 * TS * npair, odd_rows)]
                        for row0, rows in chunks:
                            lrows = rows + 1 if row0 + rows < X else rows
                            mts = mt_pool.tile([lrows, Y, 4], F32)
                            rr_dma(
                                mts[:],
                                AP(
                                    mask_h,
                                    (b * X + row0) * Y * 4,
                                    [[Y * 4, lrows], [4, Y], [1, 4]],
                                ),
                            )
                            ets = et_pool.tile([rows, Y], F32)
                            rr_dma(ets[:], edge_h.ap()[b, row0 : row0 + rows, :, 0])
                            emit_compute(mts[:, :, idx], ets[:], rows, t_ctr[0])
                            t_ctr[0] += 1
                else:
                    for b in range(Bc):
                        # one DMA for all full x-tiles' edge rows, one for the tail
                        etm = et_pool.tile([TS, 2 * npair, Y], F32)
                        getattr(nc, eng_edge).dma_start(
                            etm[:],
                            edge_h.ap()[b, 0 : 2 * TS * npair, :, 0].rearrange(
                                "(k p) y -> p k y", p=TS
                            ),
                        )
                        eto = None
                        if odd_rows:
                            eto = et_pool.tile([odd_rows, Y], F32)
                            getattr(nc, eng_edge).dma_start(
                                eto[:], edge_h.ap()[b, 2 * TS * npair : X, :, 0]
                            )
                        for m in range(npair):
                            mtp = mt_pool.tile([TS + 1, 2, Y, 4], F32)
                            getattr(nc, eng_mask).dma_start(mtp[:], mask_pair_src(b, m))
                            for j in range(2):
                                k = 2 * m + j
                                emit_compute(
                                    mtp[:, j, :, idx], etm[:, k, :], TS, b * nk + k
                                )
                        if odd_rows:
                            mto = mt_pool.tile([odd_rows, Y, 4], F32)
                            getattr(nc, eng_mask).dma_start(
                                mto[:], mask_h.ap()[b, 2 * TS * npair : X, :, :]
                            )
                            emit_compute(
                                mto[:, :, idx], eto[:], odd_rows, b * nk + nk - 1
                            )
                # total = sum_p rvec[p] * sum_t partials[p, t]
                red = c_pool.tile([128, 1], F32)
                rm = c_pool.tile([128, 1], F32)
                nc.vector.reduce_sum(red[:], partials[:], axis=mybir.AxisListType.X)
                nc.vector.tensor_mul(rm[:], red[:], rvec_t[:])
                out_ps = ps1_pool.tile([1, 1], F32)
                nc.tensor.matmul(out_ps[:], rm[:], ones_t[:], start=True, stop=True)
                out_sb = c_pool.tile([1, 1], F32)
                nc.vector.tensor_copy(out_sb[:], out_ps[:])
                nc.sync.dma_start(out_h.ap(), out_sb[:])

            if niter == 1:
                emit_iter()
            else:
                with tc.For_i(0, niter, 1):
                    emit_iter()

    nc.compile()
    return nc


def _host_consts(idx: int):
    sx, sy = SHIFTS[idx]
    smat = np.zeros((128, 128), np.float32)
    for p in range(127):
        smat[p + 1, p] = 1.0
    xs = np.arange(128)
    rvec = (
        (((xs + 4 * sx) % 8 != 7) & ((xs + 1 + 4 * sx) % 8 != 7))
        .astype(np.float32)
        .reshape(128, 1)
    )
    return smat, rvec


def _host_cvec(idx: int, Y: int):
    _, sy = SHIFTS[idx]
    ys = np.arange(Y)
    cv = (((ys + 4 * sy) % 8 != 7) & ((ys + 1 + 4 * sy) % 8 != 7)).astype(np.float32)
    return np.broadcast_to(cv, (128, Y)).copy()


def _run(mask, edge, loss_old, idx, trace=False, niter=1, n_cores=N_CORES, **build_kwargs):
    B, X, Y, _ = mask.shape
    assert B % N_CORES == 0
    Bc = B // N_CORES
    sx, sy = SHIFTS[idx]

    nc = _build_program(Bc, X, Y, idx, sy, niter=niter, **build_kwargs)
    smat, rvec = _host_consts(idx)
    cvec = _host_cvec(idx, Y)
    in_maps = [
        {
            "mask": mask[i * Bc : (i + 1) * Bc],
            "edge": edge[i * Bc : (i + 1) * Bc],
            "smat": smat,
            "rvec": rvec,
            "cvec": cvec,
        }
        for i in range(n_cores)
    ]
    res = run_bass_kernel_spmd(nc, in_maps, list(range(n_cores)), trace=trace)
    total = float(sum(float(res.results[i]["out"][0, 0]) for i in range(n_cores)))
    n_patch = ((X + 8) // 8) * ((Y + 8) // 8)
    out = np.float32(np.asarray(loss_old, dtype=np.float32) + total / (B * n_patch))
    return np.asarray(out, dtype=np.float32), res


def kernel(resized_image, mask_combined, edge_map, loss_old, mask_index):
    mask = np.ascontiguousarray(np.asarray(mask_combined, dtype=np.float32))
    edge = np.ascontiguousarray(np.asarray(edge_map, dtype=np.float32))
    idx = int(np.asarray(mask_index))
    out, _ = _run(mask, edge, loss_old, idx)
    return out



# revision 35
# speedup vs baseline: 1.2545x; 1.2545x over previous
"""Trainium2 Bass kernel for the supervoxel erode/edge loss module.

The reference divides a padded [B,X,Y] grid (pad offset 4*sx along x, 4*sy
along y) into 8x8 patches, zeroes the last row/col of the mask channel in
each patch, erodes along both patch axes and sums eroded*edge. The erode
`a*b + (1-a)*a + (1-b)*a` algebraically equals `2a - a^2` with
a = m(i)*m(i+1) (the second operand cancels), and because both the patch
shifts and the patch-boundary zeroing are local, the whole module collapses
to a global elementwise expression on the unpadded grid:

    mt(x,y) = mask[b,x,y,idx] * [(x+4sx)%8 != 7] * [(y+4sy)%8 != 7]
    ax = mt(x,y)*mt(x+1,y); ay = mt(x,y)*mt(x,y+1)   (zero past image edge)
    total = sum_b,x,y ax(2-ax) * ay(2-ay) * edge
    out = loss_old + total / (B * ((X+8)//8) * ((Y+8)//8))

With raw products ax0 = raw(x)raw(x+1), ay0 = raw(x,y)raw(x,y+1) the masks
fold out of the elementwise chain:

    contribution = ax0(2-ax0) * ay0(2-ay0) * edge * R(x) * C(y)

R(x) = [x%8 not in {6-4sx, 7-4sx}] is applied to the final per-row partial
sums, and C(y) = [y%8 not in {6-4sy, 7-4sy}] by restricting the elementwise
ops to the live columns of each 8-group (sy==0), or by one extra multiply.

x-tiles are 121 rows at stride 120 (one-row overlap so the x-neighbor
product never crosses a tile boundary; 120 % 8 == 0 keeps R per-partition
tile-invariant). DMA is the roofline: per-transfer fixed cost serializes on
the queue rings, so mask tiles are loaded two-at-a-time with one
overlapping-window DMA (~3.9 MiB each) and edge as one whole-image DMA.

Per x-tile the compute pipeline is:
    PE    : shifted = S @ v  (S = shift-by-one-row matrix; v = stride-4
            channel view of the mask tile)
    DVE   : ax0 = v*shifted, nx = (ax0-2)*ax0, ny = (ay0-2)*ay0, reduce
    Pool  : ay0 = v*v(y+1), p1 = nx*ny, p2 = p1*edge
    ((a-2)*a = -(a(2-a)); the two negations cancel in p1 = nx*ny.)

Sharding: data-parallel over batch, B/8 images per core on 8 cores; each
core returns a masked partial sum, combined on host (the mean is a single
scalar, so no device collective is needed).
"""

import sys

sys.path.insert(0, "/opt/trn_rl_repo")

import numpy as np

from concourse import bacc, bass, mybir, tile
from concourse.ap import AP
from concourse.bass_utils import run_bass_kernel_spmd

F32 = mybir.dt.float32
N_CORES = 8
TS = 120  # x-tile stride (multiple of 8 so the %8 row pattern is tile-invariant)
SHIFTS = [(0, 0), (1, 0), (0, 1), (1, 1)]


def _build_program(
    Bc: int,
    X: int,
    Y: int,
    idx: int,
    sy: int,
    niter: int = 1,
    variant: str = "full",
    dma_mode: str = "rr3",
):
    """Build the per-core Bass program. Inputs (per core):
    mask [Bc,X,Y,4] f32, edge [Bc,X,Y,1] f32, smat [128,128], rvec [128,1],
    cvec [128,Y] (used only when sy != 0). Output: out [1,1] f32 partial sum.
    niter > 1 repeats the whole computation on-device (timing only).
    """
    assert X % 8 == 0 and Y % 8 == 0
    nk = (X + TS - 1) // TS  # x-tiles per image
    nt = Bc * nk + 1  # total chunks (image 0 is split into one extra chunk)
    npair = nk // 2
    odd_rows = X - 2 * TS * npair  # rows of the trailing unpaired tile (0 if none)
    G = Y // 8
    packed = sy == 0  # live cols are j in 0..5 of every group of 8

    nc = bacc.Bacc("TRN2", target_bir_lowering=False, debug=False)
    mask_h = nc.dram_tensor("mask", [Bc, X, Y, 4], F32, kind="ExternalInput")
    edge_h = nc.dram_tensor("edge", [Bc, X, Y, 1], F32, kind="ExternalInput")
    smat_h = nc.dram_tensor("smat", [128, 128], F32, kind="ExternalInput")
    rvec_h = nc.dram_tensor("rvec", [128, 1], F32, kind="ExternalInput")
    cvec_h = nc.dram_tensor("cvec", [128, Y], F32, kind="ExternalInput")
    out_h = nc.dram_tensor("out", [1, 1], F32, kind="ExternalOutput")

    if dma_mode == "gpsimd":
        eng_mask, eng_edge = "gpsimd", "sync"
    elif dma_mode == "sync":
        eng_mask, eng_edge = "sync", "scalar"
    else:
        eng_mask, eng_edge = "scalar", "sync"

    def mask_pair_src(b, m):
        """Overlapping-window DRAM AP: [121, 2, Y, 4] where element
        (p, j, y, c) reads mask[b, 2*TS*m + TS*j + p, y, c]."""
        row = Y * 4  # elements per x-row
        off = (b * X + 2 * TS * m) * row
        ap = [[row, TS + 1], [TS * row, 2], [4, Y], [1, 4]]
        return AP(mask_h, off, ap)

    def edge_pair_src(b, m):
        """[TS, 2, Y] where (p, j, y) reads edge[b, 2*TS*m + TS*j + p, y, 0]."""
        off = (b * X + 2 * TS * m) * Y
        ap = [[Y, TS], [TS * Y, 2], [1, Y]]
        return AP(edge_h, off, ap)

    def mask_tile_src(b, k):
        """[121, Y, 4]: rows TS*k .. TS*k+121 of image b (one overlap row)."""
        row = Y * 4
        off = (b * X + TS * k) * row
        ap = [[row, TS + 1], [4, Y], [1, 4]]
        return AP(mask_h, off, ap)

    def mask_tile_src_strided(b, k):
        """[121, Y]: channel idx only, 4B elements at 16B stride."""
        row = Y * 4
        off = (b * X + TS * k) * row + idx
        ap = [[row, TS + 1], [4, Y]]
        return AP(mask_h, off, ap)

    with tile.TileContext(nc) as tc:
        with (
            tc.tile_pool(name="mt", bufs=6 if dma_mode.startswith("rr") else 2) as mt_pool,
            tc.tile_pool(name="et", bufs=6 if dma_mode.startswith("rr") else 2) as et_pool,
            tc.tile_pool(name="mtt", bufs=1) as mtt_pool,
            tc.tile_pool(name="ett", bufs=1) as ett_pool,
            tc.tile_pool(name="work", bufs=2) as w_pool,
            tc.tile_pool(name="pp", bufs=2) as p_pool,
            tc.tile_pool(name="psum", bufs=2, space="PSUM") as ps_pool,
            tc.tile_pool(name="psum1", bufs=1, space="PSUM") as ps1_pool,
            tc.tile_pool(name="const", bufs=1) as c_pool,
        ):
            smat_t = c_pool.tile([128, 128], F32)
            rvec_t = c_pool.tile([128, 1], F32)
            ones_t = c_pool.tile([128, 1], F32)
            neg1_t = c_pool.tile([128, 1], F32)
            partials = c_pool.tile([128, nt], F32)
            nc.sync.dma_start(smat_t[:], smat_h.ap())
            nc.sync.dma_start(rvec_t[:], rvec_h.ap())
            nc.gpsimd.memset(ones_t[:], 1.0)
            nc.gpsimd.memset(neg1_t[:], -1.0)
            cvec_t = None
            if not packed:
                cvec_t = c_pool.tile([128, Y], F32)
                nc.sync.dma_start(cvec_t[:], cvec_h.ap())

            def emit_compute(v, et_v, cr, t_idx):
                """v: [rows>=cr(+1), Y] stride-4 mask-channel view;
                et_v: [cr, Y] edge view; accumulates into partials[:, t_idx]."""
                rows = v.shape[0]
                if variant == "dma":
                    # timing ablation: loads only, tiny consumer so nothing is elided
                    nc.vector.reduce_sum(
                        partials[0:1, t_idx : t_idx + 1],
                        v[0:1, 0:8],
                        axis=mybir.AxisListType.X,
                    )
                    nc.gpsimd.tensor_mul(
                        partials[0:1, t_idx : t_idx + 1],
                        partials[0:1, t_idx : t_idx + 1],
                        et_v[0:1, 0:1],
                    )
                    return
                shifted = ps_pool.tile([128, Y], F32)
                if variant != "pool":
                    for c0 in range(0, Y, 512):
                        cw = min(512, Y - c0)
                        nc.tensor.matmul(
                            shifted[:, c0 : c0 + cw],
                            smat_t[0:rows, :],
                            v[:, c0 : c0 + cw],
                            start=True,
                            stop=True,
                        )

                if packed:
                    ax0 = w_pool.tile([cr, G, 6], F32)
                    ay0 = w_pool.tile([cr, G, 6], F32)
                    if variant != "full":
                        nxt = w_pool.tile([cr, G, 6], F32)
                        nyt = w_pool.tile([cr, G, 6], F32)
                    p1 = p_pool.tile([cr, G, 6], F32)
                    p2 = p_pool.tile([cr, G, 6], F32)

                    def lv(t, j0=0, j1=6):
                        return t.rearrange("p (g j) -> p g j", j=8)[:, :, j0:j1]

                    v_l = lv(v[0:cr, :])
                    v_l1 = lv(v[0:cr, :], 1, 7)  # col + 1
                    sh_l = lv(shifted[0:cr, :])
                    if variant == "full":
                        # DVE/Pool split alternates by tile parity so both
                        # engines average ~5.4us/tile, under the ~5.7us DMA
                        # slot. The final STT fuses p2 = p1*edge with its
                        # XY-reduction via accum_out (no standalone reduce).
                        nxt = w_pool.tile([cr, G, 6], F32)
                        nyt = w_pool.tile([cr, G, 6], F32)
                        nc.vector.tensor_mul(ax0[:], v_l, sh_l)
                        nc.gpsimd.tensor_mul(ay0[:], v_l, v_l1)
                        nc.vector.scalar_tensor_tensor(
                            nxt[:], ax0[:], 2.0, ax0[:],
                            op0=mybir.AluOpType.subtract, op1=mybir.AluOpType.mult,
                        )
                        nc.vector.scalar_tensor_tensor(
                            nyt[:], ay0[:], 2.0, ay0[:],
                            op0=mybir.AluOpType.subtract, op1=mybir.AluOpType.mult,
                        )
                        nc.gpsimd.tensor_mul(p1[:], nxt[:], nyt[:])
                        # p2 = p1*edge: Pool/DVE alternate by tile parity
                        p2_eng = nc.gpsimd if t_idx % 2 == 0 else nc.vector
                        p2_eng.tensor_mul(p2[:], p1[:], lv(et_v))
                        # PE reduce: acc[0,c] += sum_p rvec[p] * p2[p,c];
                        # the rvec stationary folds the row mask in for free
                        p2f = p2.rearrange("p g j -> p (g j)")
                        first = t_idx == 0
                        last = t_idx == last_chunk
                        nc.tensor.matmul(
                            acc_ps[0][:, :], rvec_t[0:cr, 0:1], p2f[:, 0:512],
                            start=first, stop=last,
                        )
                        nc.tensor.matmul(
                            acc_ps[1][:, :], rvec_t[0:cr, 0:1], p2f[:, 512:768],
                            start=first, stop=last,
                        )
                        return
                    if variant == "dve":
                        nc.vector.tensor_mul(ax0[:], v_l, sh_l)
                        nc.vector.scalar_tensor_tensor(
                            nxt[:], ax0[:], 2.0, ax0[:],
                            op0=mybir.AluOpType.subtract, op1=mybir.AluOpType.mult,
                        )
                        nc.vector.scalar_tensor_tensor(
                            nyt[:], nxt[:], 2.0, nxt[:],
                            op0=mybir.AluOpType.subtract, op1=mybir.AluOpType.mult,
                        )
                        nc.vector.reduce_sum(
                            partials[0:cr, t_idx : t_idx + 1], nyt[:],
                            axis=mybir.AxisListType.XY,
                        )
                        return
                    if variant == "pool":
                        nc.gpsimd.tensor_mul(ay0[:], v_l, v_l1)
                        nc.gpsimd.tensor_mul(p1[:], ay0[:], ay0[:])
                        nc.gpsimd.tensor_mul(p2[:], p1[:], lv(et_v))
                        nc.vector.reduce_sum(
                            partials[0:cr, t_idx : t_idx + 1], p2[:],
                            axis=mybir.AxisListType.XY,
                        )
                        return
                    # ax0 = v * (v shifted one row); ay0 = v * (v shifted one col)
                    nc.vector.tensor_mul(ax0[:], v_l, sh_l)
                    nc.gpsimd.tensor_mul(ay0[:], v_l, v_l1)
                    # n = (a - 2) * a = -e; the negations cancel in the product
                    nc.vector.scalar_tensor_tensor(
                        nxt[:], ax0[:], 2.0, ax0[:],
                        op0=mybir.AluOpType.subtract, op1=mybir.AluOpType.mult,
                    )
                    nc.vector.scalar_tensor_tensor(
                        nyt[:], ay0[:], 2.0, ay0[:],
                        op0=mybir.AluOpType.subtract, op1=mybir.AluOpType.mult,
                    )
                    nc.gpsimd.tensor_mul(p1[:], nxt[:], nyt[:])
                    nc.gpsimd.tensor_mul(p2[:], p1[:], lv(et_v))
                    nc.vector.reduce_sum(
                        partials[0:cr, t_idx : t_idx + 1], p2[:],
                        axis=mybir.AxisListType.XY,
                    )
                else:
                    W = Y - 1
                    ax0 = w_pool.tile([cr, Y], F32)
                    ay0 = w_pool.tile([cr, Y], F32)
                    nxt = w_pool.tile([cr, Y], F32)
                    nyt = w_pool.tile([cr, Y], F32)
                    p1 = p_pool.tile([cr, Y], F32)
                    p2 = p_pool.tile([cr, Y], F32)
                    nc.vector.tensor_mul(ax0[:, 0:W], v[0:cr, 0:W], shifted[0:cr, 0:W])
                    nc.gpsimd.tensor_mul(ay0[:, 0:W], v[0:cr, 0:W], v[0:cr, 1:Y])
                    # fold the column mask into ay0 (C is 0/1 so e_y picks it up)
                    nc.gpsimd.tensor_mul(ay0[:, 0:W], ay0[:, 0:W], cvec_t[0:cr, 0:W])
                    nc.vector.scalar_tensor_tensor(
                        nxt[:, 0:W], ax0[:, 0:W], 2.0, ax0[:, 0:W],
                        op0=mybir.AluOpType.subtract, op1=mybir.AluOpType.mult,
                    )
                    nc.vector.scalar_tensor_tensor(
                        nyt[:, 0:W], ay0[:, 0:W], 2.0, ay0[:, 0:W],
                        op0=mybir.AluOpType.subtract, op1=mybir.AluOpType.mult,
                    )
                    nc.gpsimd.tensor_mul(p1[:, 0:W], nxt[:, 0:W], nyt[:, 0:W])
                    nc.gpsimd.tensor_mul(p2[:, 0:W], p1[:, 0:W], et_v[:, 0:W])
                    nc.vector.reduce_sum(
                        partials[0:cr, t_idx : t_idx + 1], p2[:, 0:W],
                        axis=mybir.AxisListType.X,
                    )

            ring_cycle = {
                "rr1": (nc.sync,),
                "rr2": (nc.sync, nc.scalar),
                "rr3": (nc.sync, nc.scalar, nc.gpsimd),
                "rr3s": (nc.sync, nc.scalar, nc.gpsimd),
                "rrs": (nc.sync, nc.scalar, nc.gpsimd),
            }.get(dma_mode, (nc.sync, nc.scalar))
            # 4KB packets measured slightly faster than 16KB
            dma_kw = {"max_dma_last_dim": 1024} if dma_mode in ("rr3s",) else {}
            ring_bytes = [0] * len(ring_cycle)

            def rr_dma(dst, src):
                """Send each chunk to the currently least-loaded DMA ring
                (greedy byte balancing - keeps all rings finishing together).
                The SP/Activation HWDGE rings never stall behind engine work;
                qPoolDynamic (SWDGE) adds a third stream."""
                i = ring_bytes.index(min(ring_bytes))
                ring_bytes[i] += dst.size()
                return ring_cycle[i].dma_start(dst, src, **dma_kw)

            pe_acc = variant == "full" and packed and dma_mode.startswith("rr")
            last_chunk = nt - 1 if dma_mode.startswith("rr") else Bc * nk - 1
            acc_ps = [None, None]

            def emit_iter():
                if pe_acc:
                    acc_ps[0] = ps1_pool.tile([1, 512], F32, name="acc0")
                    acc_ps[1] = ps1_pool.tile([1, G * 6 - 512], F32, name="acc1")
                else:
                    nc.vector.memset(partials[:], 0.0)
                if dma_mode.startswith("rr"):
                    # chunked rows, round-robin across the DMA rings. The
                    # first two chunks of image 0 are small (64 rows) so the
                    # first compute can start ~15us in instead of waiting for
                    # a full 120-row tile to win its share of the rings.
                    # Every chunk start stays == 0 mod 8 (R row-mask phase).
                    t_ctr = [0]
                    for b in range(Bc):
                        if b == 0:
                            chunks = [(0, 64), (64, 64)]
                            chunks += [(128 + TS * i, TS) for i in range(7)]
                            chunks += [(968, X - 968)]
                        else:
                            chunks = [(TS * i, TS) for i in range(2 * npair)]
                            if odd_rows:
                                chunks += [(2 * TS * npair, odd_rows)]
                        for row0, rows in chunks:
                            lrows = rows + 1 if row0 + rows < X else rows
                            mts = mt_pool.tile([lrows, Y, 4], F32)
                            rr_dma(
                                mts[:],
                                AP(
                                    mask_h,
                                    (b * X + row0) * Y * 4,
                                    [[Y * 4, lrows], [4, Y], [1, 4]],
                                ),
                            )
                            ets = et_pool.tile([rows, Y], F32)
                            rr_dma(ets[:], edge_h.ap()[b, row0 : row0 + rows, :, 0])
                            emit_compute(mts[:, :, idx], ets[:], rows, t_ctr[0])
                            t_ctr[0] += 1
                else:
                    for b in range(Bc):
                        # one DMA for all full x-tiles' edge rows, one for the tail
                        etm = et_pool.tile([TS, 2 * npair, Y], F32)
                        getattr(nc, eng_edge).dma_start(
                            etm[:],
                            edge_h.ap()[b, 0 : 2 * TS * npair, :, 0].rearrange(
                                "(k p) y -> p k y", p=TS
                            ),
                        )
                        eto = None
                        if odd_rows:
                            eto = et_pool.tile([odd_rows, Y], F32)
                            getattr(nc, eng_edge).dma_start(
                                eto[:], edge_h.ap()[b, 2 * TS * npair : X, :, 0]
                            )
                        for m in range(npair):
                            mtp = mt_pool.tile([TS + 1, 2, Y, 4], F32)
                            getattr(nc, eng_mask).dma_start(mtp[:], mask_pair_src(b, m))
                            for j in range(2):
                                k = 2 * m + j
                                emit_compute(
                                    mtp[:, j, :, idx], etm[:, k, :], TS, b * nk + k
                                )
                        if odd_rows:
                            mto = mt_pool.tile([odd_rows, Y, 4], F32)
                            getattr(nc, eng_mask).dma_start(
                                mto[:], mask_h.ap()[b, 2 * TS * npair : X, :, :]
                            )
                            emit_compute(
                                mto[:, :, idx], eto[:], odd_rows, b * nk + nk - 1
                            )
                if pe_acc:
                    # acc already folds rvec and the partition sum; just sum
                    # the 768 accumulated packed columns
                    sb_acc = c_pool.tile([1, G * 6], F32)
                    nc.vector.tensor_copy(sb_acc[:, 0:512], acc_ps[0][:])
                    nc.vector.tensor_copy(sb_acc[:, 512 : G * 6], acc_ps[1][:])
                    out_sb = c_pool.tile([1, 1], F32)
                    nc.vector.reduce_sum(
                        out_sb[:], sb_acc[:], axis=mybir.AxisListType.X
                    )
                    nc.sync.dma_start(out_h.ap(), out_sb[:])
                else:
                    # total = sum_p rvec[p] * sum_t partials[p, t]
                    red = c_pool.tile([128, 1], F32)
                    rm = c_pool.tile([128, 1], F32)
                    nc.vector.reduce_sum(red[:], partials[:], axis=mybir.AxisListType.X)
                    nc.vector.tensor_mul(rm[:], red[:], rvec_t[:])
                    out_ps = ps1_pool.tile([1, 1], F32)
                    nc.tensor.matmul(out_ps[:], rm[:], ones_t[:], start=True, stop=True)
                    out_sb = c_pool.tile([1, 1], F32)
                    nc.vector.tensor_copy(out_sb[:], out_ps[:])
                    nc.sync.dma_start(out_h.ap(), out_sb[:])

            if niter == 1:
                emit_iter()
            else:
                with tc.For_i(0, niter, 1):
                    emit_iter()

    nc.compile()
    return nc


def _host_consts(idx: int):
    sx, sy = SHIFTS[idx]
    smat = np.zeros((128, 128), np.float32)
    for p in range(127):
        smat[p + 1, p] = 1.0
    xs = np.arange(128)
    rvec = (
        (((xs + 4 * sx) % 8 != 7) & ((xs + 1 + 4 * sx) % 8 != 7))
        .astype(np.float32)
        .reshape(128, 1)
    )
    return smat, rvec


def _host_cvec(idx: int, Y: int):
    _, sy = SHIFTS[idx]
    ys = np.arange(Y)
    cv = (((ys + 4 * sy) % 8 != 7) & ((ys + 1 + 4 * sy) % 8 != 7)).astype(np.float32)
    return np.broadcast_to(cv, (128, Y)).copy()


def _run(mask, edge, loss_old, idx, trace=False, niter=1, n_cores=N_CORES, **build_kwargs):
    B, X, Y, _ = mask.shape
    assert B % N_CORES == 0
    Bc = B // N_CORES
    sx, sy = SHIFTS[idx]

    nc = _build_program(Bc, X, Y, idx, sy, niter=niter, **build_kwargs)
    smat, rvec = _host_consts(idx)
    cvec = _host_cvec(idx, Y)
    in_maps = [
        {
            "mask": mask[i * Bc : (i + 1) * Bc],
            "edge": edge[i * Bc : (i + 1) * Bc],
            "smat": smat,
            "rvec": rvec,
            "cvec": cvec,
        }
        for i in range(n_cores)
    ]
    res = run_bass_kernel_spmd(nc, in_maps, list(range(n_cores)), trace=trace)
    total = float(sum(float(res.results[i]["out"][0, 0]) for i in range(n_cores)))
    n_patch = ((X + 8) // 8) * ((Y + 8) // 8)
    out = np.float32(np.asarray(loss_old, dtype=np.float32) + total / (B * n_patch))
    return np.asarray(out, dtype=np.float32), res


def kernel(resized_image, mask_combined, edge_map, loss_old, mask_index):
    mask = np.ascontiguousarray(np.asarray(mask_combined, dtype=np.float32))
    edge = np.ascontiguousarray(np.asarray(edge_map, dtype=np.float32))
    idx = int(np.asarray(mask_index))
    out, _ = _run(mask, edge, loss_old, idx)
    return out



# revision 41
# speedup vs baseline: 1.2819x; 1.0218x over previous
"""Trainium2 Bass kernel for the supervoxel erode/edge loss module.

The reference divides a padded [B,X,Y] grid (pad offset 4*sx along x, 4*sy
along y) into 8x8 patches, zeroes the last row/col of the mask channel in
each patch, erodes along both patch axes and sums eroded*edge. The erode
`a*b + (1-a)*a + (1-b)*a` algebraically equals `2a - a^2` with
a = m(i)*m(i+1) (the second operand cancels), and because both the patch
shifts and the patch-boundary zeroing are local, the whole module collapses
to a global elementwise expression on the unpadded grid:

    mt(x,y) = mask[b,x,y,idx] * [(x+4sx)%8 != 7] * [(y+4sy)%8 != 7]
    ax = mt(x,y)*mt(x+1,y); ay = mt(x,y)*mt(x,y+1)   (zero past image edge)
    total = sum_b,x,y ax(2-ax) * ay(2-ay) * edge
    out = loss_old + total / (B * ((X+8)//8) * ((Y+8)//8))

With raw products ax0 = raw(x)raw(x+1), ay0 = raw(x,y)raw(x,y+1) the masks
fold out of the elementwise chain:

    contribution = ax0(2-ax0) * ay0(2-ay0) * edge * R(x) * C(y)

R(x) = [x%8 not in {6-4sx, 7-4sx}] is applied to the final per-row partial
sums, and C(y) = [y%8 not in {6-4sy, 7-4sy}] by restricting the elementwise
ops to the live columns of each 8-group (sy==0), or by one extra multiply.

x-tiles are 121 rows at stride 120 (one-row overlap so the x-neighbor
product never crosses a tile boundary; 120 % 8 == 0 keeps R per-partition
tile-invariant). DMA is the roofline: per-transfer fixed cost serializes on
the queue rings, so mask tiles are loaded two-at-a-time with one
overlapping-window DMA (~3.9 MiB each) and edge as one whole-image DMA.

Per x-tile the compute pipeline is:
    PE    : shifted = S @ v  (S = shift-by-one-row matrix; v = stride-4
            channel view of the mask tile)
    DVE   : ax0 = v*shifted, nx = (ax0-2)*ax0, ny = (ay0-2)*ay0, reduce
    Pool  : ay0 = v*v(y+1), p1 = nx*ny, p2 = p1*edge
    ((a-2)*a = -(a(2-a)); the two negations cancel in p1 = nx*ny.)

Sharding: data-parallel over batch, B/8 images per core on 8 cores; each
core returns a masked partial sum, combined on host (the mean is a single
scalar, so no device collective is needed).
"""

import sys

sys.path.insert(0, "/opt/trn_rl_repo")

import numpy as np

from concourse import bacc, bass, mybir, tile
from concourse.ap import AP
from concourse.bass_utils import run_bass_kernel_spmd

F32 = mybir.dt.float32
BF16 = mybir.dt.bfloat16
N_CORES = 8
TS = 120  # x-tile stride (multiple of 8 so the %8 row pattern is tile-invariant)
SHIFTS = [(0, 0), (1, 0), (0, 1), (1, 1)]


def _build_program(
    Bc: int,
    X: int,
    Y: int,
    idx: int,
    sy: int,
    niter: int = 1,
    variant: str = "full",
    dma_mode: str = "rr3",
):
    """Build the per-core Bass program. Inputs (per core):
    mask [Bc,X,Y,4] f32, edge [Bc,X,Y,1] f32, smat [128,128], rvec [128,1],
    cvec [128,Y] (used only when sy != 0). Output: out [1,1] f32 partial sum.
    niter > 1 repeats the whole computation on-device (timing only).
    """
    assert X % 8 == 0 and Y % 8 == 0
    nk = (X + TS - 1) // TS  # x-tiles per image
    nt = Bc * nk + 1  # total chunks (image 0 is split into one extra chunk)
    npair = nk // 2
    odd_rows = X - 2 * TS * npair  # rows of the trailing unpaired tile (0 if none)
    G = Y // 8
    packed = sy == 0  # live cols are j in 0..5 of every group of 8

    nc = bacc.Bacc("TRN2", target_bir_lowering=False, debug=False)
    mask_h = nc.dram_tensor("mask", [Bc, X, Y, 4], F32, kind="ExternalInput")
    edge_h = nc.dram_tensor("edge", [Bc, X, Y, 1], F32, kind="ExternalInput")
    smat_h = nc.dram_tensor("smat", [128, 128], F32, kind="ExternalInput")
    rvec_h = nc.dram_tensor("rvec", [128, 1], F32, kind="ExternalInput")
    cvec_h = nc.dram_tensor("cvec", [128, Y], F32, kind="ExternalInput")
    out_h = nc.dram_tensor("out", [1, 1], F32, kind="ExternalOutput")

    if dma_mode == "gpsimd":
        eng_mask, eng_edge = "gpsimd", "sync"
    elif dma_mode == "sync":
        eng_mask, eng_edge = "sync", "scalar"
    else:
        eng_mask, eng_edge = "scalar", "sync"

    def mask_pair_src(b, m):
        """Overlapping-window DRAM AP: [121, 2, Y, 4] where element
        (p, j, y, c) reads mask[b, 2*TS*m + TS*j + p, y, c]."""
        row = Y * 4  # elements per x-row
        off = (b * X + 2 * TS * m) * row
        ap = [[row, TS + 1], [TS * row, 2], [4, Y], [1, 4]]
        return AP(mask_h, off, ap)

    def edge_pair_src(b, m):
        """[TS, 2, Y] where (p, j, y) reads edge[b, 2*TS*m + TS*j + p, y, 0]."""
        off = (b * X + 2 * TS * m) * Y
        ap = [[Y, TS], [TS * Y, 2], [1, Y]]
        return AP(edge_h, off, ap)

    def mask_tile_src(b, k):
        """[121, Y, 4]: rows TS*k .. TS*k+121 of image b (one overlap row)."""
        row = Y * 4
        off = (b * X + TS * k) * row
        ap = [[row, TS + 1], [4, Y], [1, 4]]
        return AP(mask_h, off, ap)

    def mask_tile_src_strided(b, k):
        """[121, Y]: channel idx only, 4B elements at 16B stride."""
        row = Y * 4
        off = (b * X + TS * k) * row + idx
        ap = [[row, TS + 1], [4, Y]]
        return AP(mask_h, off, ap)

    with tile.TileContext(nc) as tc:
        with (
            tc.tile_pool(name="mt", bufs=6 if dma_mode.startswith("rr") else 2) as mt_pool,
            tc.tile_pool(name="et", bufs=6 if dma_mode.startswith("rr") else 2) as et_pool,
            tc.tile_pool(name="mtt", bufs=1) as mtt_pool,
            tc.tile_pool(name="ett", bufs=1) as ett_pool,
            tc.tile_pool(name="work", bufs=2) as w_pool,
            tc.tile_pool(name="pp", bufs=2) as p_pool,
            tc.tile_pool(name="psum", bufs=2, space="PSUM") as ps_pool,
            tc.tile_pool(name="psum1", bufs=1, space="PSUM") as ps1_pool,
            tc.tile_pool(name="const", bufs=1) as c_pool,
        ):
            smat_t = c_pool.tile([128, 128], F32)
            rvec_t = c_pool.tile([128, 1], F32)
            ones_t = c_pool.tile([128, 1], F32)
            neg1_t = c_pool.tile([128, 1], F32)
            partials = c_pool.tile([128, nt], F32)
            rvec_bf = c_pool.tile([128, 1], BF16)
            nc.sync.dma_start(smat_t[:], smat_h.ap())
            nc.sync.dma_start(rvec_t[:], rvec_h.ap())
            nc.vector.tensor_copy(rvec_bf[:], rvec_t[:])
            nc.gpsimd.memset(ones_t[:], 1.0)
            nc.gpsimd.memset(neg1_t[:], -1.0)
            cvec_t = None
            if not packed:
                cvec_t = c_pool.tile([128, Y], F32)
                nc.sync.dma_start(cvec_t[:], cvec_h.ap())

            def emit_compute(v, et_v, cr, t_idx):
                """v: [rows>=cr(+1), Y] stride-4 mask-channel view;
                et_v: [cr, Y] edge view; accumulates into partials[:, t_idx]."""
                rows = v.shape[0]
                if variant == "dma":
                    # timing ablation: loads only, tiny consumer so nothing is elided
                    nc.vector.reduce_sum(
                        partials[0:1, t_idx : t_idx + 1],
                        v[0:1, 0:8],
                        axis=mybir.AxisListType.X,
                    )
                    nc.gpsimd.tensor_mul(
                        partials[0:1, t_idx : t_idx + 1],
                        partials[0:1, t_idx : t_idx + 1],
                        et_v[0:1, 0:1],
                    )
                    return
                shifted = ps_pool.tile([128, Y], F32)
                if variant != "pool":
                    for c0 in range(0, Y, 512):
                        cw = min(512, Y - c0)
                        nc.tensor.matmul(
                            shifted[:, c0 : c0 + cw],
                            smat_t[0:rows, :],
                            v[:, c0 : c0 + cw],
                            start=True,
                            stop=True,
                        )

                if packed:
                    if variant != "full":
                        ax0 = w_pool.tile([cr, G, 6], F32)
                        ay0 = w_pool.tile([cr, G, 6], F32)
                        nxt = w_pool.tile([cr, G, 6], F32)
                        nyt = w_pool.tile([cr, G, 6], F32)
                        p1 = p_pool.tile([cr, G, 6], F32)
                        p2 = p_pool.tile([cr, G, 6], F32)

                    def lv(t, j0=0, j1=6):
                        return t.rearrange("p (g j) -> p g j", j=8)[:, :, j0:j1]

                    v_l = lv(v[0:cr, :])
                    v_l1 = lv(v[0:cr, :], 1, 7)  # col + 1
                    sh_l = lv(shifted[0:cr, :])
                    if variant == "full":
                        # bf16 elementwise chain: ax0/ay0 downcast on write,
                        # the rest run at 2x DVE rate. The edge tile arrives
                        # bf16 via a casting SWDGE DMA. Reductions stay f32
                        # (PE accumulates PSUM in f32).
                        bax0 = w_pool.tile([cr, G, 6], BF16)
                        bay0 = w_pool.tile([cr, G, 6], BF16)
                        nxt = w_pool.tile([cr, G, 6], BF16)
                        nyt = w_pool.tile([cr, G, 6], BF16)
                        bp1 = p_pool.tile([cr, G, 6], BF16)
                        bp2 = p_pool.tile([cr, G, 6], BF16)
                        nc.vector.tensor_mul(bax0[:], v_l, sh_l)
                        nc.gpsimd.tensor_mul(bay0[:], v_l, v_l1)
                        nc.vector.scalar_tensor_tensor(
                            nxt[:], bax0[:], 2.0, bax0[:],
                            op0=mybir.AluOpType.subtract, op1=mybir.AluOpType.mult,
                        )
                        nc.vector.scalar_tensor_tensor(
                            nyt[:], bay0[:], 2.0, bay0[:],
                            op0=mybir.AluOpType.subtract, op1=mybir.AluOpType.mult,
                        )
                        nc.gpsimd.tensor_mul(bp1[:], nxt[:], nyt[:])
                        nc.vector.tensor_mul(bp2[:], bp1[:], lv(et_v))
                        p2 = bp2
                        # PE reduce: acc[0,c] += sum_p rvec[p] * p2[p,c];
                        # the rvec stationary folds the row mask in for free
                        p2f = p2.rearrange("p g j -> p (g j)")
                        first = t_idx == 0
                        last = t_idx == last_chunk
                        nc.tensor.matmul(
                            acc_ps[0][:, :], rvec_bf[0:cr, 0:1], p2f[:, 0:512],
                            start=first, stop=last,
                        )
                        nc.tensor.matmul(
                            acc_ps[1][:, :], rvec_bf[0:cr, 0:1], p2f[:, 512:768],
                            start=first, stop=last,
                        )
                        return
                    if variant == "dve":
                        nc.vector.tensor_mul(ax0[:], v_l, sh_l)
                        nc.vector.scalar_tensor_tensor(
                            nxt[:], ax0[:], 2.0, ax0[:],
                            op0=mybir.AluOpType.subtract, op1=mybir.AluOpType.mult,
                        )
                        nc.vector.scalar_tensor_tensor(
                            nyt[:], nxt[:], 2.0, nxt[:],
                            op0=mybir.AluOpType.subtract, op1=mybir.AluOpType.mult,
                        )
                        nc.vector.reduce_sum(
                            partials[0:cr, t_idx : t_idx + 1], nyt[:],
                            axis=mybir.AxisListType.XY,
                        )
                        return
                    if variant == "pool":
                        nc.gpsimd.tensor_mul(ay0[:], v_l, v_l1)
                        nc.gpsimd.tensor_mul(p1[:], ay0[:], ay0[:])
                        nc.gpsimd.tensor_mul(p2[:], p1[:], lv(et_v))
                        nc.vector.reduce_sum(
                            partials[0:cr, t_idx : t_idx + 1], p2[:],
                            axis=mybir.AxisListType.XY,
                        )
                        return
                    # ax0 = v * (v shifted one row); ay0 = v * (v shifted one col)
                    nc.vector.tensor_mul(ax0[:], v_l, sh_l)
                    nc.gpsimd.tensor_mul(ay0[:], v_l, v_l1)
                    # n = (a - 2) * a = -e; the negations cancel in the product
                    nc.vector.scalar_tensor_tensor(
                        nxt[:], ax0[:], 2.0, ax0[:],
                        op0=mybir.AluOpType.subtract, op1=mybir.AluOpType.mult,
                    )
                    nc.vector.scalar_tensor_tensor(
                        nyt[:], ay0[:], 2.0, ay0[:],
                        op0=mybir.AluOpType.subtract, op1=mybir.AluOpType.mult,
                    )
                    nc.gpsimd.tensor_mul(p1[:], nxt[:], nyt[:])
                    nc.gpsimd.tensor_mul(p2[:], p1[:], lv(et_v))
                    nc.vector.reduce_sum(
                        partials[0:cr, t_idx : t_idx + 1], p2[:],
                        axis=mybir.AxisListType.XY,
                    )
                else:
                    W = Y - 1
                    ax0 = w_pool.tile([cr, Y], F32)
                    ay0 = w_pool.tile([cr, Y], F32)
                    nxt = w_pool.tile([cr, Y], F32)
                    nyt = w_pool.tile([cr, Y], F32)
                    p1 = p_pool.tile([cr, Y], F32)
                    p2 = p_pool.tile([cr, Y], F32)
                    nc.vector.tensor_mul(ax0[:, 0:W], v[0:cr, 0:W], shifted[0:cr, 0:W])
                    nc.gpsimd.tensor_mul(ay0[:, 0:W], v[0:cr, 0:W], v[0:cr, 1:Y])
                    # fold the column mask into ay0 (C is 0/1 so e_y picks it up)
                    nc.gpsimd.tensor_mul(ay0[:, 0:W], ay0[:, 0:W], cvec_t[0:cr, 0:W])
                    nc.vector.scalar_tensor_tensor(
                        nxt[:, 0:W], ax0[:, 0:W], 2.0, ax0[:, 0:W],
                        op0=mybir.AluOpType.subtract, op1=mybir.AluOpType.mult,
                    )
                    nc.vector.scalar_tensor_tensor(
                        nyt[:, 0:W], ay0[:, 0:W], 2.0, ay0[:, 0:W],
                        op0=mybir.AluOpType.subtract, op1=mybir.AluOpType.mult,
                    )
                    nc.gpsimd.tensor_mul(p1[:, 0:W], nxt[:, 0:W], nyt[:, 0:W])
                    nc.gpsimd.tensor_mul(p2[:, 0:W], p1[:, 0:W], et_v[:, 0:W])
                    nc.vector.reduce_sum(
                        partials[0:cr, t_idx : t_idx + 1], p2[:, 0:W],
                        axis=mybir.AxisListType.X,
                    )

            ring_cycle = {
                "rr1": (nc.sync,),
                "rr2": (nc.sync, nc.scalar),
                "rr3": (nc.sync, nc.scalar, nc.gpsimd),
                "rr3s": (nc.sync, nc.scalar, nc.gpsimd),
                "rrs": (nc.sync, nc.scalar, nc.gpsimd),
            }.get(dma_mode, (nc.sync, nc.scalar))
            # 4KB packets measured slightly faster than 16KB
            dma_kw = {"max_dma_last_dim": 1024} if dma_mode in ("rr3s",) else {}
            ring_bytes = [0] * len(ring_cycle)

            def rr_dma(dst, src):
                """Send each chunk to the currently least-loaded DMA ring
                (greedy byte balancing - keeps all rings finishing together).
                The SP/Activation HWDGE rings never stall behind engine work;
                qPoolDynamic (SWDGE) adds a third stream."""
                i = ring_bytes.index(min(ring_bytes))
                ring_bytes[i] += dst.size()
                return ring_cycle[i].dma_start(dst, src, **dma_kw)

            pe_acc = variant == "full" and packed and dma_mode.startswith("rr")
            last_chunk = nt - 1 if dma_mode.startswith("rr") else Bc * nk - 1
            acc_ps = [None, None]

            def emit_iter():
                if pe_acc:
                    acc_ps[0] = ps1_pool.tile([1, 512], F32, name="acc0")
                    acc_ps[1] = ps1_pool.tile([1, G * 6 - 512], F32, name="acc1")
                else:
                    nc.vector.memset(partials[:], 0.0)
                if dma_mode.startswith("rr"):
                    # chunked rows, round-robin across the DMA rings. The
                    # first two chunks of image 0 are small (64 rows) so the
                    # first compute can start ~15us in instead of waiting for
                    # a full 120-row tile to win its share of the rings.
                    # Every chunk start stays == 0 mod 8 (R row-mask phase).
                    t_ctr = [0]
                    for b in range(Bc):
                        if b == 0:
                            chunks = [(0, 64), (64, 64)]
                            chunks += [(128 + TS * i, TS) for i in range(7)]
                            chunks += [(968, X - 968)]
                        else:
                            chunks = [(TS * i, TS) for i in range(2 * npair)]
                            if odd_rows:
                                chunks += [(2 * TS * npair, odd_rows)]
                        for row0, rows in chunks:
                            lrows = rows + 1 if row0 + rows < X else rows
                            mts = mt_pool.tile([lrows, Y, 4], F32)
                            rr_dma(
                                mts[:],
                                AP(
                                    mask_h,
                                    (b * X + row0) * Y * 4,
                                    [[Y * 4, lrows], [4, Y], [1, 4]],
                                ),
                            )
                            e_src = edge_h.ap()[b, row0 : row0 + rows, :, 0]
                            if pe_acc:
                                # bf16 edge via casting SWDGE DMA (gp ring)
                                ets = et_pool.tile([rows, Y], BF16)
                                gp_i = ring_cycle.index(nc.gpsimd)
                                ring_bytes[gp_i] += e_src.size()
                                nc.gpsimd.dma_start(ets[:], e_src, **dma_kw)
                            else:
                                ets = et_pool.tile([rows, Y], F32)
                                rr_dma(ets[:], e_src)
                            emit_compute(mts[:, :, idx], ets[:], rows, t_ctr[0])
                            t_ctr[0] += 1
                else:
                    for b in range(Bc):
                        # one DMA for all full x-tiles' edge rows, one for the tail
                        etm = et_pool.tile([TS, 2 * npair, Y], F32)
                        getattr(nc, eng_edge).dma_start(
                            etm[:],
                            edge_h.ap()[b, 0 : 2 * TS * npair, :, 0].rearrange(
                                "(k p) y -> p k y", p=TS
                            ),
                        )
                        eto = None
                        if odd_rows:
                            eto = et_pool.tile([odd_rows, Y], F32)
                            getattr(nc, eng_edge).dma_start(
                                eto[:], edge_h.ap()[b, 2 * TS * npair : X, :, 0]
                            )
                        for m in range(npair):
                            mtp = mt_pool.tile([TS + 1, 2, Y, 4], F32)
                            getattr(nc, eng_mask).dma_start(mtp[:], mask_pair_src(b, m))
                            for j in range(2):
                                k = 2 * m + j
                                emit_compute(
                                    mtp[:, j, :, idx], etm[:, k, :], TS, b * nk + k
                                )
                        if odd_rows:
                            mto = mt_pool.tile([odd_rows, Y, 4], F32)
                            getattr(nc, eng_mask).dma_start(
                                mto[:], mask_h.ap()[b, 2 * TS * npair : X, :, :]
                            )
                            emit_compute(
                                mto[:, :, idx], eto[:], odd_rows, b * nk + nk - 1
                            )
                if pe_acc:
                    # acc already folds rvec and the partition sum; just sum
                    # the 768 accumulated packed columns
                    sb_acc = c_pool.tile([1, G * 6], F32)
                    nc.vector.tensor_copy(sb_acc[:, 0:512], acc_ps[0][:])
                    nc.vector.tensor_copy(sb_acc[:, 512 : G * 6], acc_ps[1][:])
                    out_sb = c_pool.tile([1, 1], F32)
                    nc.vector.reduce_sum(
                        out_sb[:], sb_acc[:], axis=mybir.AxisListType.X
                    )
                    nc.sync.dma_start(out_h.ap(), out_sb[:])
                else:
                    # total = sum_p rvec[p] * sum_t partials[p, t]
                    red = c_pool.tile([128, 1], F32)
                    rm = c_pool.tile([128, 1], F32)
                    nc.vector.reduce_sum(red[:], partials[:], axis=mybir.AxisListType.X)
                    nc.vector.tensor_mul(rm[:], red[:], rvec_t[:])
                    out_ps = ps1_pool.tile([1, 1], F32)
                    nc.tensor.matmul(out_ps[:], rm[:], ones_t[:], start=True, stop=True)
                    out_sb = c_pool.tile([1, 1], F32)
                    nc.vector.tensor_copy(out_sb[:], out_ps[:])
                    nc.sync.dma_start(out_h.ap(), out_sb[:])

            if niter == 1:
                emit_iter()
            else:
                with tc.For_i(0, niter, 1):
                    emit_iter()

    nc.compile()
    return nc


def _host_consts(idx: int):
    sx, sy = SHIFTS[idx]
    smat = np.zeros((128, 128), np.float32)
    for p in range(127):
        smat[p + 1, p] = 1.0
    xs = np.arange(128)
    rvec = (
        (((xs + 4 * sx) % 8 != 7) & ((xs + 1 + 4 * sx) % 8 != 7))
        .astype(np.float32)
        .reshape(128, 1)
    )
    return smat, rvec


def _host_cvec(idx: int, Y: int):
    _, sy = SHIFTS[idx]
    ys = np.arange(Y)
    cv = (((ys + 4 * sy) % 8 != 7) & ((ys + 1 + 4 * sy) % 8 != 7)).astype(np.float32)
    return np.broadcast_to(cv, (128, Y)).copy()


def _run(mask, edge, loss_old, idx, trace=False, niter=1, n_cores=N_CORES, **build_kwargs):
    B, X, Y, _ = mask.shape
    assert B % N_CORES == 0
    Bc = B // N_CORES
    sx, sy = SHIFTS[idx]

    nc = _build_program(Bc, X, Y, idx, sy, niter=niter, **build_kwargs)
    smat, rvec = _host_consts(idx)
    cvec = _host_cvec(idx, Y)
    in_maps = [
        {
            "mask": mask[i * Bc : (i + 1) * Bc],
            "edge": edge[i * Bc : (i + 1) * Bc],
            "smat": smat,
            "rvec": rvec,
            "cvec": cvec,
        }
        for i in range(n_cores)
    ]
    res = run_bass_kernel_spmd(nc, in_maps, list(range(n_cores)), trace=trace)
    total = float(sum(float(res.results[i]["out"][0, 0]) for i in range(n_cores)))
    n_patch = ((X + 8) // 8) * ((Y + 8) // 8)
    out = np.float32(np.asarray(loss_old, dtype=np.float32) + total / (B * n_patch))
    return np.asarray(out, dtype=np.float32), res


def kernel(resized_image, mask_combined, edge_map, loss_old, mask_index):
    mask = np.ascontiguousarray(np.asarray(mask_combined, dtype=np.float32))
    edge = np.ascontiguousarray(np.asarray(edge_map, dtype=np.float32))
    idx = int(np.asarray(mask_index))
    out, _ = _run(mask, edge, loss_old, idx)
    return out

